# revision 42
# baseline (speedup 1.0000x reference)
"""BiLSTM-CRF forward loss on 8 TRN2 NeuronCores (Bass/Tile), v3.

v2 "pair plan": each core runs ONE LSTM direction for 8 sequences (direction
encoded in per-core data: backward cores get time-reversed tokens and
k/gate-permuted weights; the SPMD instruction stream is identical). Paired
cores (c, c+4) exchange h histories between layers via pairwise AllGather +
indirect-DMA partner-slot reads. Recurrent weights are fp8 (x32, scale folded
out via activation `scale`), cutting LDWEIGHTS to ~26ns/pair.

v3 on top: gates reordered [g,i,f,o] host-side so the per-step nonlinearity
chain is short and fused (3 adds / 4 activations, single PSUM tile); h is
additionally stored in position-contiguous layouts (histC for own GEMMs, mir
for the partner) so the L1-input / highway GEMMs stream at full rate;
embedding transposes overlap gx0; CRF scan uses batched chain pairs with a
leaner renorm and numerator/aux ops interleaved into its latency gaps.
"""
import os
import sys

import numpy as np

try:
    import concourse  # noqa: F401
except ImportError:  # pragma: no cover
    sys.path.insert(0, "/opt/trn_rl_repo")

import ml_dtypes
from contextlib import ExitStack

import concourse.bass as bass
import concourse.bacc as bacc
import concourse.mybir as mybir
import concourse.tile as tile
from concourse.bass_utils import run_bass_kernel_spmd

F32 = mybir.dt.float32
BF16 = mybir.dt.bfloat16
FP8 = mybir.dt.float8e4
U8 = mybir.dt.uint8
I32 = mybir.dt.int32
AF = mybir.ActivationFunctionType
ALU = mybir.AluOpType
AX = mybir.AxisListType

B, S, E, H, T, V = 32, 256, 256, 512, 17, 50000
NC = 8
NSEQ = 8              # sequences per pair-group (one direction per core)
TOK = NSEQ * S        # 2048 local tokens, flat = 256*q + s
G = 4 * H
GC = G // 128         # 16 gate chunks, device order [g, i, f, o]
KH = H // 128
KE = E // 128
K1 = 2 * H // 128
RENORM = 8
NREN = (S - 1) // RENORM
SPL = 64              # exchange split: slots [SPL:S] shipped at t==192
USE_FP8 = True
WSCALE = 32.0 if USE_FP8 else 1.0
WDT = FP8 if USE_FP8 else BF16
NFILL = int(os.environ.get("NFILL", "0"))
FILMV = int(os.environ.get("FILMV", "64"))

_CACHE = {}


def _build_nc():
    nc = bacc.Bacc(None, target_bir_lowering=False, num_devices=NC)
    d = {}
    P = nc.declare_dram_parameter
    d["xt"] = P("xt", [128, KE * TOK], WDT, isOutput=False)
    d["wih0T"] = P("wih0T", [KE, 128, G], WDT, isOutput=False)
    d["whh0T"] = P("whh0T", [KH, 128, G], WDT, isOutput=False)
    d["wih1T"] = P("wih1T", [K1, 128, G], WDT, isOutput=False)
    d["whh1T"] = P("whh1T", [KH, 128, G], WDT, isOutput=False)
    d["hwT"] = P("hwT", [2, K1, 128, 2 * H], WDT, isOutput=False)
    d["fcwT"] = P("fcwT", [128, K1 * T], BF16, isOutput=False)
    d["aux128"] = P("aux128", [128, 48], F32, isOutput=False)
    d["ident"] = P("ident", [128, 128], BF16, isOutput=False)
    d["aux17"] = P("aux17", [T, 2220], F32, isOutput=False)
    d["mren"] = P("mren", [1, NREN * NSEQ], F32, isOutput=False)
    d["vm"] = P("vm", [1, TOK], U8, isOutput=False)
    d["msel"] = P("msel", [T, TOK], U8, isOutput=False)
    d["prow"] = P("prow", [128, 1], I32, isOutput=False)
    out_d = P("out", [4, NSEQ], F32, isOutput=True)

    with tile.TileContext(nc) as tc, ExitStack() as ctx:
        pp = ctx.enter_context(tc.tile_pool(name="persist", bufs=1))
        wp = ctx.enter_context(tc.tile_pool(name="wts", bufs=1))
        sp = ctx.enter_context(tc.tile_pool(name="small", bufs=2))
        op = ctx.enter_context(tc.tile_pool(name="once", bufs=1))
        ps = ctx.enter_context(tc.tile_pool(name="psum", bufs=2, space="PSUM"))
        dp = ctx.enter_context(tc.tile_pool(name="dram", bufs=4, space="DRAM"))

        dma = nc.sync.dma_start

        # ---- weights first (prefetch), then small tables -----------------------
        wih_sb = wp.tile([128, K1, G], WDT, tag="wih", name="wih0")
        for k in range(KE):
            dma(wih_sb[:, k, :], d["wih0T"][k])
        whh_sb = wp.tile([128, KH, G], WDT, tag="whh", name="whh0")
        for k in range(KH):
            nc.scalar.dma_start(whh_sb[:, k, :], d["whh0T"][k])
        fcw_sb = pp.tile([128, K1, T], BF16, tag="fcw")
        dma(fcw_sb[:], d["fcwT"][:].rearrange("p (k t) -> p k t", k=K1))
        aux128_sb = pp.tile([128, 48], F32, tag="aux128")
        dma(aux128_sb[:], d["aux128"][:])
        aux17_sb = pp.tile([T, 2220], F32, tag="aux17")
        dma(aux17_sb[:], d["aux17"][:])
        mren_sb = pp.tile([1, NREN, NSEQ], F32, tag="mren")
        dma(mren_sb[:], d["mren"][:].rearrange("o (k b) -> o k b", k=NREN))
        vm_sb = pp.tile([1, NSEQ, S], U8, tag="vm")
        dma(vm_sb[:], d["vm"][:].rearrange("o (b s) -> o b s", b=NSEQ))
        msel_sb = pp.tile([T, NSEQ, S], U8, tag="msel")
        dma(msel_sb[:], d["msel"][:].rearrange("t (b s) -> t b s", b=NSEQ))
        prow_sb = pp.tile([128, 1], I32, tag="prow")
        dma(prow_sb[:], d["prow"][:])
        ident_sb = pp.tile([128, 128], BF16, tag="ident")
        dma(ident_sb[:], d["ident"][:])

        def b0v(c):
            return aux128_sb[:, c : c + 1]

        def b1v(c):
            return aux128_sb[:, 16 + c : 16 + c + 1]

        def hwbv(w, c):
            return aux128_sb[:, 32 + 8 * w + c : 32 + 8 * w + c + 1]

        trans_sb = aux17_sb[:, 0:T]
        svec_sb = aux17_sb[:, T : T + 1]
        evec_sb = aux17_sb[:, T + 1 : T + 2]
        fcb_sb = aux17_sb[:, T + 2 : T + 3]
        oh_sb = aux17_sb[:, 20 : 20 + TOK].rearrange("t (b s) -> t b s", b=NSEQ)
        cp_base = 20 + TOK
        s0e_sb = aux17_sb[:, cp_base + NSEQ * T : cp_base + NSEQ * T + 2 * NSEQ]

        ones_t = pp.tile([T, 1], F32, tag="onesT")
        nc.vector.memset(ones_t[:], 1.0)
        ones_1t = pp.tile([1, T], F32, tag="ones1T")
        nc.vector.memset(ones_1t[:], 1.0)

        # ---- embedding: host pre-gathered + transposed -------------------------
        XT = pp.tile([128, KE, TOK], WDT, tag="XT")
        dma(XT[:], d["xt"][:].rearrange("p (k t) -> p k t", k=KE))
        gx = pp.tile([128, GC, NSEQ, S], BF16, tag="gx", name="gx0")
        for bp in range(NSEQ // 2):
            for c in range(GC):
                pt = ps.tile([128, 512], F32, tag="mm")
                nc.tensor.matmul(
                    pt[:],
                    wih_sb[:, 0:2, bass.ts(c, 128)],
                    XT[:, 0:2, bass.ts(bp, 512)],
                    start=True,
                    stop=True,
                    perf_mode=mybir.MatmulPerfMode.DoubleRow,
                )
                nc.vector.tensor_scalar(
                    out=gx[:, c, 2 * bp : 2 * bp + 2, :],
                    in0=pt[:].rearrange("p (b s) -> p b s", b=2),
                    scalar1=b0v(c), scalar2=None,
                    op0=ALU.add,
                )

        # ---- recurrence + exchange --------------------------------------------
        RG = [[0, 4], [1, 5], [2, 6], [3, 7]]
        NS = 1.0 / WSCALE
        fil_mv = pp.tile([128, 256], BF16, tag="filmv")
        nc.vector.memset(fil_mv[:], 0.0)

        CHS = 16  # s-positions per exchange chunk (128 KB per contribution)

        def lstm_layer(layer, whh_sb, gxl):
            histC = pp.tile(
                [128, KH, NSEQ, S], FP8, tag="histC", name=f"histC{layer}"
            )
            mir = pp.tile([128, KH, NSEQ, S], FP8, tag="mir", name=f"mir{layer}")
            part = pp.tile(
                [128, KH, NSEQ, S], FP8, tag="part", name=f"part{layer}"
            )

            CWF = KH * NSEQ * CHS

            def ship(i):
                bi = dp.tile([128, CWF], FP8, tag="bi", name=f"bi{layer}_{i}")
                bo = dp.tile([256, CWF], FP8, tag="bo", name=f"bo{layer}_{i}")
                stgo = sp.tile([128, KH * NSEQ, CHS], FP8, tag="stgo", bufs=1)
                nc.vector.tensor_copy(
                    stgo[:], mir[:, :, :, CHS * i : CHS * (i + 1)]
                )
                dma(bi[:], stgo[:].rearrange("p a s -> p (a s)"))
                nc.gpsimd.collective_compute(
                    "AllGather", ALU.bypass, replica_groups=RG,
                    ins=[bi[:].opt()], outs=[bo[:].opt()],
                )
                stgi = sp.tile([128, KH * NSEQ, CHS], FP8, tag="stgi", bufs=1)
                nc.gpsimd.indirect_dma_start(
                    out=stgi[:].rearrange("p a s -> p (a s)"),
                    out_offset=None,
                    in_=bo[:],
                    in_offset=bass.IndirectOffsetOnAxis(ap=prow_sb[:, 0:1], axis=0),
                )
                nc.vector.tensor_copy(
                    part[:, :, :, CHS * i : CHS * (i + 1)], stgi[:]
                )

            cst = {}
            for par in range(2):
                cst[par] = pp.tile(
                    [128, KH, NSEQ], F32, tag=f"cst{par}", name=f"c{layer}p{par}"
                )
            nc.vector.memset(cst[0][:], 0.0)
            # per-gate-type PSUM tiles so an activation reading one type never
            # blocks matmuls accumulating into another type; single parity —
            # the preload's WAR on the previous step's activation read is
            # satisfied by placement (g/i/f preloads after this step's stream,
            # o preload at the next step's head, where h is awaited anyway)
            pg = [
                ps.tile(
                    [128, 4, NSEQ], F32, tag=f"pg{typ}", bufs=1,
                    name=f"pg{layer}_{typ}",
                )
                for typ in range(4)
            ]
            def preload(tt, typs):
                for typ in typs:
                    nc.tensor.matmul(
                        pg[typ][:],
                        ident_sb[:],
                        gxl[:, 4 * typ : 4 * typ + 4, :, tt],
                        start=True,
                        stop=False,
                        skip_group_check=True,
                    )

            preload(0, (0, 1, 2, 3))
            for t in range(S):
                if t >= CHS and t % CHS == 0:
                    ship((S - t) // CHS)
                sig = sp.tile([128, GC, NSEQ], F32, tag="sig")
                if t > 0:
                    preload(t, (3,))
                    for gc in range(GC):
                        for k in range(KH):
                            nc.tensor.matmul(
                                pg[gc // 4][:, gc % 4, :],
                                whh_sb[:, k, bass.ts(gc, 128)],
                                hh_prev[:, k, :],
                                start=False,
                                stop=(k == KH - 1),
                                skip_group_check=True,
                            )
                        if gc == 3:
                            nc.scalar.activation(
                                sig[:, 0:4, :], pg[0][:], AF.Tanh, scale=NS
                            )
                        elif gc == 7:
                            nc.scalar.activation(
                                sig[:, 4:8, :], pg[1][:], AF.Sigmoid, scale=NS
                            )
                        elif gc == 11:
                            nc.scalar.activation(
                                sig[:, 8:12, :], pg[2][:], AF.Sigmoid, scale=NS
                            )
                    nc.scalar.activation(
                        sig[:, 12:16, :], pg[3][:], AF.Sigmoid, scale=NS
                    )
                else:
                    nc.scalar.activation(
                        sig[:, 0:4, :], pg[0][:], AF.Tanh, scale=NS
                    )
                    for typ in range(1, 4):
                        nc.scalar.activation(
                            sig[:, 4 * typ : 4 * typ + 4, :], pg[typ][:],
                            AF.Sigmoid, scale=NS,
                        )
                if NFILL:
                    pfil = ps.tile([128, 512], F32, tag="mm")
                    for _ in range(NFILL):
                        nc.tensor.matmul(
                            pfil[:, 0:FILMV],
                            fil_mv[:, 0:128],
                            fil_mv[:, 0:FILMV],
                            start=True,
                            stop=True,
                            skip_group_check=True,
                        )
                if t + 1 < S:
                    preload(t + 1, (0, 1, 2))
                c_old, c_new = cst[t % 2], cst[1 - t % 2]
                ig = sp.tile([128, KH, NSEQ], F32, tag="ig")
                nc.vector.tensor_mul(ig[:], sig[:, 4:8, :], sig[:, 0:4, :])
                if t > 0:
                    nc.vector.tensor_mul(c_new[:], sig[:, 8:12, :], c_old[:])
                    nc.vector.tensor_add(c_new[:], c_new[:], ig[:])
                else:
                    nc.vector.tensor_copy(c_new[:], ig[:])
                th = sp.tile([128, KH, NSEQ], F32, tag="th")
                nc.scalar.activation(th[:], c_new[:], AF.Tanh)
                hh = sp.tile([128, KH, NSEQ], FP8, tag="hh")
                nc.vector.tensor_mul(hh[:], sig[:, 12:16, :], th[:])
                nc.gpsimd.tensor_copy(histC[:, :, :, t], hh[:])
                nc.gpsimd.tensor_copy(mir[:, :, :, S - 1 - t], hh[:])
                hh_prev = hh
            ship(0)
            return histC, part

        histC0, part0 = lstm_layer(0, whh_sb, gx)
        _KDBG = os.environ.get("KDBG")

        # ---- L1 input GEMM -----------------------------------------------------
        wih_sb = wp.tile([128, K1, G], WDT, tag="wih", name="wih1")
        for k in range(K1):
            dma(wih_sb[:, k, :], d["wih1T"][k])

        def x_slice(histC, part, k, b):
            if k < KH:
                return histC[:, k, b, :]
            return part[:, k - KH, b, :]

        def x_pair(histC, part, j, bp):
            # k-pair j x seq-pair bp -> [128, 2, 512] moving (seq dims merge)
            if j < KH // 2:
                return histC[:, 2 * j : 2 * j + 2, 2 * bp : 2 * bp + 2, :]
            jj = j - KH // 2
            return part[:, 2 * jj : 2 * jj + 2, 2 * bp : 2 * bp + 2, :]

        gx1 = pp.tile([128, GC, NSEQ, S], BF16, tag="gx", name="gx1")
        for c in range(GC):
            for bp in range(NSEQ // 2):
                pt = ps.tile([128, 512], F32, tag="mm")
                for j in range(K1 // 2):
                    nc.tensor.matmul(
                        pt[:],
                        wih_sb[:, 2 * j : 2 * j + 2, bass.ts(c, 128)],
                        x_pair(histC0, part0, j, bp),
                        start=(j == 0),
                        stop=(j == K1 // 2 - 1),
                        perf_mode=mybir.MatmulPerfMode.DoubleRow,
                    )
                nc.vector.tensor_scalar(
                    out=gx1[:, c, 2 * bp : 2 * bp + 2, :],
                    in0=pt[:].rearrange("p (b s) -> p b s", b=2),
                    scalar1=b1v(c), scalar2=None,
                    op0=ALU.add,
                )

        if _KDBG:
            dbgt = sp.tile([1, NSEQ], F32, tag="dbg")
            nc.vector.tensor_copy(dbgt[:, 0:1], part0[0:1, 1, 0, 100:101])
            nc.vector.tensor_copy(dbgt[:, 1:2], part0[0:1, 0, 0, 10:11])
            nc.vector.tensor_copy(dbgt[:, 2:3], part0[0:1, 0, 0, 63:64])
            nc.vector.tensor_copy(dbgt[:, 3:4], histC0[0:1, 0, 0, 10:11])
            nc.vector.tensor_copy(dbgt[:, 4:5], wih_sb[0:1, 0, 0:1])
            nc.vector.tensor_copy(dbgt[:, 5:6], gx1[0:1, 0, 0, 10:11])
            nc.vector.tensor_copy(dbgt[:, 6:7], gx1[0:1, 0, 0, 100:101])
            nc.vector.tensor_copy(dbgt[:, 7:8], gx1[0:1, 15, 7, 200:201])
            dma(out_d[3:4, :], dbgt[:])

        whh_sb = wp.tile([128, KH, G], WDT, tag="whh", name="whh1")
        for k in range(KH):
            nc.scalar.dma_start(whh_sb[:, k, :], d["whh1T"][k])
        histC1, part1 = lstm_layer(1, whh_sb, gx1)

        # ---- highway + fc ------------------------------------------------------
        hw_sb = pp.tile([128, 2, K1, 2 * H], WDT, tag="gx", name="hw")
        for w in range(2):
            for k in range(K1):
                dma(hw_sb[:, w, k, :], d["hwT"][w, k])
        x2 = wp.tile([128, K1, TOK], BF16, tag="wih", name="x2")

        def xs2(histC, part, c, bp):
            if c < KH:
                sl = histC[:, c, 2 * bp : 2 * bp + 2, :]
            else:
                sl = part[:, c - KH, 2 * bp : 2 * bp + 2, :]
            return sl.rearrange("p b s -> p (b s)")

        for c in range(8):
            for bp in range(NSEQ // 2):
                ptt = ps.tile([128, 512], F32, tag="mm")
                pth = ps.tile([128, 512], F32, tag="mm")
                for j in range(K1 // 2):
                    nc.tensor.matmul(
                        ptt[:], hw_sb[:, 0, 2 * j : 2 * j + 2, bass.ts(c, 128)],
                        x_pair(histC1, part1, j, bp),
                        start=(j == 0), stop=(j == K1 // 2 - 1),
                        perf_mode=mybir.MatmulPerfMode.DoubleRow,
                    )
                for j in range(K1 // 2):
                    nc.tensor.matmul(
                        pth[:], hw_sb[:, 1, 2 * j : 2 * j + 2, bass.ts(c, 128)],
                        x_pair(histC1, part1, j, bp),
                        start=(j == 0), stop=(j == K1 // 2 - 1),
                        perf_mode=mybir.MatmulPerfMode.DoubleRow,
                    )
                tg = sp.tile([128, 512], F32, tag="tg")
                nc.scalar.activation(
                    tg[:], ptt[:], AF.Sigmoid, bias=hwbv(0, c), scale=NS
                )
                rl = sp.tile([128, 512], F32, tag="rl")
                nc.scalar.activation(
                    rl[:], pth[:], AF.Relu, bias=hwbv(1, c), scale=NS
                )
                dd_ = sp.tile([128, 512], F32, tag="dd")
                nc.vector.tensor_sub(dd_[:], rl[:], xs2(histC1, part1, c, bp))
                nc.vector.tensor_mul(dd_[:], tg[:], dd_[:])
                nc.vector.tensor_add(
                    x2[:, c, bass.ts(bp, 512)], dd_[:], xs2(histC1, part1, c, bp)
                )

        logits = pp.tile([T, NSEQ, S], F32, tag="hist", name="logits")
        for bp in range(NSEQ // 2):
            pt = ps.tile([128, 512], F32, tag="mm")
            for k in range(K1):
                nc.tensor.matmul(
                    pt[:T, :], fcw_sb[:, k, :], x2[:, k, bass.ts(bp, 512)],
                    start=(k == 0), stop=(k == K1 - 1),
                )
            nc.scalar.activation(
                logits[:, 2 * bp : 2 * bp + 2, :],
                pt[:T, :].rearrange("p (b s) -> p b s", b=2),
                AF.Identity, bias=fcb_sb,
            )

        # ---- CRF + numerator + aux --------------------------------------------
        expEm = pp.tile([T, NSEQ, S], F32, tag="XT", name="expEm")
        nc.scalar.activation(expEm[:], logits[:], AF.Exp)
        expT = op.tile([T, T], F32, tag="expT")
        nc.scalar.activation(expT[:], trans_sb, AF.Exp)
        expS = op.tile([T, 1], F32, tag="expS")
        nc.scalar.activation(expS[:], svec_sb, AF.Exp)
        expE = op.tile([T, 1], F32, tag="expE")
        nc.scalar.activation(expE[:], evec_sb, AF.Exp)

        # numerator / aux emitters, interleaved into CRF latency gaps
        emm = pp.tile([T, NSEQ, S], F32, tag="mir", name="emm")
        empart = sp.tile([T, NSEQ], F32, tag="empart")
        nv = sp.tile([T, NSEQ], F32, tag="nv")
        ev = sp.tile([T, NSEQ], F32, tag="ev")
        lse = pp.tile([1, NSEQ, S], F32, tag="part", name="lse")
        num_sb = sp.tile([1, NSEQ], F32, tag="num")
        aux_sb = sp.tile([1, NSEQ], F32, tag="aux")
        trp = op.tile([T, T], F32, tag="trp")
        trr = sp.tile([T, NSEQ], F32, tag="trr")

        fills = []

        def emit_fills():
            fills.append(lambda: nc.vector.tensor_mul(emm[:], logits[:], oh_sb))
            fills.append(
                lambda: nc.vector.reduce_sum(empart[:], emm[:], axis=AX.X)
            )
            fills.append(
                lambda: nc.vector.tensor_scalar(
                    out=nv[:], in0=s0e_sb[:, 0:NSEQ], scalar1=svec_sb,
                    scalar2=None, op0=ALU.mult,
                )
            )
            fills.append(
                lambda: nc.vector.tensor_scalar(
                    out=ev[:], in0=s0e_sb[:, NSEQ : 2 * NSEQ], scalar1=evec_sb,
                    scalar2=None, op0=ALU.mult,
                )
            )
            fills.append(lambda: nc.vector.tensor_add(nv[:], nv[:], ev[:]))
            fills.append(lambda: nc.vector.tensor_add(nv[:], nv[:], empart[:]))
            for bb in range(NSEQ):
                fills.append(
                    lambda bb=bb: nc.vector.tensor_mul(
                        trp[:],
                        aux17_sb[:, cp_base + T * bb : cp_base + T * (bb + 1)],
                        trans_sb,
                    )
                )
                fills.append(
                    lambda bb=bb: nc.vector.reduce_sum(
                        trr[:, bb : bb + 1], trp[:], axis=AX.X
                    )
                )
            for hlf in range(4):
                def aux_lse(hlf=hlf):
                    psE = ps.tile([1, 512], F32, tag="small")
                    nc.tensor.matmul(
                        psE[:, :512], ones_t[:],
                        expEm[:, 2 * hlf : 2 * hlf + 2, :],
                        start=True, stop=True,
                    )
                    nc.scalar.activation(
                        lse[:, 2 * hlf : 2 * hlf + 2, :],
                        psE[:].rearrange("o (b s) -> o b s", b=2),
                        AF.Ln,
                    )
                fills.append(aux_lse)

        emit_fills()

        afin = pp.tile([T, NSEQ], F32, tag="afin")
        lacc = {}
        for ch in range(2):
            for par in range(2):
                lacc[(ch, par)] = pp.tile(
                    [1, 4], F32, tag=f"lacc{ch}{par}", name=f"lacc{ch}{par}"
                )
            nc.vector.memset(lacc[(ch, 0)][:], 0.0)
        ap = ctx.enter_context(tc.tile_pool(name="crf", bufs=4))

        A = {}
        for ch in range(2):
            sl = slice(4 * ch, 4 * ch + 4)
            A[ch] = ap.tile([T, 4], F32, tag=f"A{ch}", name=f"A{ch}")
            nc.vector.tensor_scalar(
                out=A[ch][:], in0=expEm[:, sl, 0], scalar1=expS[:, 0:1],
                scalar2=None, op0=ALU.mult,
            )
        nren_seen = 0
        for t in range(1, S):
            for ch in range(2):
                sl = slice(4 * ch, 4 * ch + 4)
                pt = ps.tile([128, 4], F32, tag="mm")
                nc.tensor.matmul(pt[:T, :], expT[:], A[ch][:], start=True, stop=True)
                A[ch] = ap.tile([T, 4], F32, tag=f"A{ch}", name=f"A{ch}")
                nc.vector.tensor_mul(A[ch][:], pt[:T, :], expEm[:, sl, t])
            if fills:
                fills.pop(0)()
            if t % RENORM == 0:
                for ch in range(2):
                    sl = slice(4 * ch, 4 * ch + 4)
                    psS = ps.tile([1, 512], F32, tag="small")
                    nc.tensor.matmul(
                        psS[:, :4], ones_t[:], A[ch][:], start=True, stop=True
                    )
                    Sr = ap.tile([1, 4], F32, tag=f"Sr{ch}", name=f"Sr{ch}")
                    nc.vector.reciprocal(Sr[:], psS[:, :4])
                    lnS = ap.tile([1, 4], F32, tag=f"lnS{ch}", name=f"lnS{ch}")
                    nc.scalar.activation(lnS[:], psS[:, :4], AF.Ln)
                    pB = ps.tile([128, 4], F32, tag="mm")
                    nc.tensor.matmul(pB[:T, :], ones_1t[:], Sr[:], start=True, stop=True)
                    A2 = ap.tile([T, 4], F32, tag=f"A{ch}", name=f"A{ch}")
                    nc.vector.tensor_mul(A2[:], A[ch][:], pB[:T, :])
                    A[ch] = A2
                    nc.vector.tensor_mul(lnS[:], lnS[:], mren_sb[:, nren_seen, sl])
                    old, new = lacc[(ch, nren_seen % 2)], lacc[(ch, 1 - nren_seen % 2)]
                    nc.vector.tensor_add(new[:], old[:], lnS[:])
                nren_seen += 1
            if t >= S // 2 - 1:
                for ch in range(2):
                    sl = slice(4 * ch, 4 * ch + 4)
                    nc.vector.copy_predicated(
                        afin[:, sl], msel_sb[:, sl, t], A[ch][:]
                    )
        for f in fills:
            f()

        ae = op.tile([T, NSEQ], F32, tag="ae")
        nc.vector.tensor_scalar(
            out=ae[:], in0=afin[:], scalar1=expE[:, 0:1], scalar2=None, op0=ALU.mult
        )
        psZ = ps.tile([1, 512], F32, tag="small")
        nc.tensor.matmul(psZ[:, :NSEQ], ones_t[:], ae[:], start=True, stop=True)
        logZ = sp.tile([1, NSEQ], F32, tag="logZ")
        nc.scalar.activation(logZ[:], psZ[:, :NSEQ], AF.Ln)
        for ch in range(2):
            sl = slice(4 * ch, 4 * ch + 4)
            nc.vector.tensor_add(
                logZ[:, sl], logZ[:, sl], lacc[(ch, nren_seen % 2)][:]
            )

        # numerator wrap-up (trr columns were filled per-seq during the scan)
        nc.vector.tensor_add(nv[:], nv[:], trr[:])
        psN = ps.tile([1, 512], F32, tag="small")
        nc.tensor.matmul(psN[:, :NSEQ], ones_t[:], nv[:], start=True, stop=True)
        nc.vector.tensor_copy(num_sb[:], psN[:, :NSEQ])

        nc.vector.tensor_mul(lse[:], lse[:], vm_sb)
        lsum = sp.tile([1, NSEQ], F32, tag="lsum")
        nc.vector.reduce_sum(lsum[:], lse[:], axis=AX.X)
        psM = ps.tile([1, 512], F32, tag="small")
        nc.tensor.matmul(psM[:, :NSEQ], ones_t[:], empart[:], start=True, stop=True)
        nc.vector.tensor_sub(aux_sb[:], lsum[:], psM[:, :NSEQ])

        dma(out_d[0:1, :], num_sb[:])
        dma(out_d[1:2, :], logZ[:])
        dma(out_d[2:3, :], aux_sb[:])

    nc.compile()
    return nc


PG = np.concatenate(
    [np.arange(2 * H, 3 * H), np.arange(0, H), np.arange(H, 2 * H),
     np.arange(3 * H, 4 * H)]
)  # PyTorch gate rows [i,f,g,o] -> device order [g,i,f,o]


def _prep_maps(inputs):
    bf = ml_dtypes.bfloat16
    f8 = ml_dtypes.float8_e4m3
    x = np.asarray(inputs["x"]).astype(np.int32)
    tags = np.asarray(inputs["tags"]).astype(np.int32)
    emb = np.asarray(inputs["emb"], np.float32)
    sc = np.float32(WSCALE)

    def t_chunks(w, perm=None):
        # (G, K) -> (K//128, 128, G) with gate-row reorder; optional input perm
        w = np.asarray(w, np.float32)[PG]
        K = w.shape[1]
        wT = np.ascontiguousarray(w.T).reshape(K // 128, 128, G)
        if perm is not None:
            wT = wT[perm]
        return wT

    XS = np.float32(8.0)  # xt is scaled x8 for fp8 range; wih0 x4 keeps net x32
    wih0 = [
        t_chunks((sc / XS) * np.asarray(inputs["w_ih_l0"], np.float32)[dd])
        for dd in range(2)
    ]
    whh0 = [t_chunks(sc * np.asarray(inputs["w_hh_l0"], np.float32)[dd]) for dd in range(2)]
    whh1 = [t_chunks(sc * np.asarray(inputs["w_hh_l1"], np.float32)[dd]) for dd in range(2)]
    swap = [4, 5, 6, 7, 0, 1, 2, 3]
    wih1 = {}
    for cls in range(2):
        perm = None if cls == 0 else swap
        wih1[cls] = [
            t_chunks(sc * np.asarray(inputs["w_ih_l1"], np.float32)[dd], perm)
            for dd in range(2)
        ]

    PH = np.arange(2 * H)
    PH_swap = np.concatenate([PH[H:], PH[:H]])
    hw_t = np.asarray(inputs["hw_t_w"], np.float32)
    hw_h = np.asarray(inputs["hw_h_w"], np.float32)
    hw_tb = np.asarray(inputs["hw_t_b"], np.float32)
    hw_hb = np.asarray(inputs["hw_h_b"], np.float32)
    fcw = np.asarray(inputs["fc_w"], np.float32)
    hwT, fcwT, hwb = {}, {}, {}
    for cls in range(2):
        pr = PH if cls == 0 else PH_swap
        ht = hw_t[np.ix_(pr, pr)]
        hh = hw_h[np.ix_(pr, pr)]
        hwT[cls] = np.stack(
            [
                (sc * ht.T).reshape(K1, 128, 2 * H),
                (sc * hh.T).reshape(K1, 128, 2 * H),
            ]
        ).astype(f8 if USE_FP8 else bf)
        fcwT[cls] = (
            np.ascontiguousarray(fcw[:, pr].T)
            .reshape(K1, 128, T)
            .transpose(1, 0, 2)
            .reshape(128, K1 * T)
            .astype(bf)
        )
        hwb[cls] = (
            hw_tb[pr].reshape(8, 128).T,
            hw_hb[pr].reshape(8, 128).T,
        )

    b0 = (sc * np.asarray(inputs["b_l0"], np.float32))[:, PG].reshape(2, GC, 128)
    b1 = (sc * np.asarray(inputs["b_l1"], np.float32))[:, PG].reshape(2, GC, 128)
    trans = np.asarray(inputs["crf_trans"], np.float32)
    svec = np.asarray(inputs["crf_start"], np.float32)
    evec = np.asarray(inputs["crf_end"], np.float32)
    fcb = np.asarray(inputs["fc_b"], np.float32)

    valid = tags != 0
    lengths = (x != 0).sum(1)

    maps = []
    for core in range(NC):
        cls = 0 if core < 4 else 1
        dd = cls
        g = core % 4
        sl = slice(g * NSEQ, (g + 1) * NSEQ)
        xl_nat, tl, vl, ll = x[sl], tags[sl], valid[sl], lengths[sl]
        xl = xl_nat if cls == 0 else xl_nat[:, ::-1]
        flat = np.ascontiguousarray(xl).reshape(-1)
        xe = emb[flat] * XS  # (TOK, E) f32, x8 for fp8 range
        xt = (
            np.ascontiguousarray(xe.T)
            .reshape(KE, 128, TOK)
            .transpose(1, 0, 2)
            .reshape(128, KE * TOK)
            .astype(f8 if USE_FP8 else bf)
        )

        aux128 = np.zeros((128, 48), np.float32)
        aux128[:, 0:16] = b0[dd].reshape(16, 128).T
        aux128[:, 16:32] = b1[dd].reshape(16, 128).T
        aux128[:, 32:40] = hwb[cls][0]
        aux128[:, 40:48] = hwb[cls][1]

        jj = np.arange(T)
        oh = (tl[None, :, :] == jj[:, None, None]) & vl[None, :, :]
        oh_tags = oh.reshape(T, TOK).astype(np.float32)
        cp = np.zeros((T, NSEQ, T), np.float32)
        prev, cur = tl[:, :-1], tl[:, 1:]
        vstep = vl[:, 1:]
        for b in range(NSEQ):
            np.add.at(cp[:, b, :], (prev[b][vstep[b]], cur[b][vstep[b]]), 1.0)
        s0e = np.zeros((T, 2 * NSEQ), np.float32)
        for b in range(NSEQ):
            s0e[tl[b, 0], b] = 1.0
            s0e[tl[b, ll[b] - 1], NSEQ + b] = 1.0
        msel = np.zeros((NSEQ, S), np.float32)
        for b in range(NSEQ):
            msel[b, ll[b] - 1] = 1.0
        msel = np.broadcast_to(msel.reshape(1, TOK), (T, TOK)).astype(np.uint8)
        mren = np.zeros((NREN, NSEQ), np.float32)
        for k in range(NREN):
            mren[k] = (RENORM * (k + 1) <= ll - 1).astype(np.float32)
        aux17 = np.zeros((T, 2220), np.float32)
        aux17[:, 0:T] = trans
        aux17[:, T] = svec
        aux17[:, T + 1] = evec
        aux17[:, T + 2] = fcb
        aux17[:, 20 : 20 + TOK] = oh_tags
        aux17[:, 20 + TOK : 20 + TOK + NSEQ * T] = cp.reshape(T, NSEQ * T)
        aux17[:, 20 + TOK + NSEQ * T : 20 + TOK + NSEQ * T + 2 * NSEQ] = s0e
        vm = vl.reshape(1, TOK).astype(np.uint8)
        prow = np.arange(128, dtype=np.int32).reshape(128, 1)
        if cls == 0:
            prow = prow + 128

        maps.append(
            dict(
                xt=xt,
                wih0T=wih0[dd].astype(f8 if USE_FP8 else bf),
                whh0T=whh0[dd].astype(f8 if USE_FP8 else bf),
                wih1T=wih1[cls][dd].astype(f8 if USE_FP8 else bf),
                whh1T=whh1[dd].astype(f8 if USE_FP8 else bf),
                hwT=hwT[cls],
                fcwT=fcwT[cls],
                aux128=aux128,
                aux17=aux17,
                mren=mren.reshape(1, -1),
                vm=vm,
                msel=msel,
                prow=prow,
                ident=np.eye(128, dtype=bf),
            )
        )
    return maps, valid


TRACE = {}


def kernel(**inputs):
    if "nc" not in _CACHE:
        _CACHE["nc"] = _build_nc()
    nc = _CACHE["nc"]
    maps, valid = _prep_maps(inputs)
    kw = {}
    if TRACE.get("on"):
        kw = dict(trace=True, tmpdir=TRACE.get("dir"), trace_cores=[0])
    res = run_bass_kernel_spmd(nc, maps, list(range(NC)), **kw)
    TRACE["last"] = res
    outs = [res.results[i]["out"] for i in range(4)]
    num = np.concatenate([o[0] for o in outs])
    logZ = np.concatenate([o[1] for o in outs])
    aux = np.concatenate([o[2] for o in outs])
    crf_loss = -np.mean(num - logZ, dtype=np.float32)
    aux_loss = np.float32(aux.sum()) / np.float32(max(valid.sum(), 1))
    return np.float32(crf_loss + np.float32(0.1) * aux_loss)

